# revision 20
# baseline (speedup 1.0000x reference)
"""Trainium2 Bass kernel for nn_Attention_57080115364834.

Reference computation (B=4, C=512, H=W=64, N=H*W=4096 tokens):
    t = x.reshape(b, c, n).swapaxes(1, 2)          # (b, n, c)
    q, k, v = t@Wq.T+bq, t@Wk.T+bk, t@Wv.T+bv
    attn = softmax(q @ k.T / sqrt(c))              # (b, n, n)
    out = (attn @ v) @ Wo.T + bo                   # (b, n, c)
    return out.reshape(b, c, h, w)                 # raw view, no permute

Sharding: 8 cores = 4 batches x 2 token-halves. Each core projects
K^T/VW only for its OWN 2048-token half, then the pair exchanges halves
with an AllGather (replica groups [[0,1],[2,3],[4,5],[6,7]]) so each
core attends its 2048 queries over all 4096 keys. Both cores see K/V
in identical gathered order, so the math is exactly the reference's.

Host-side algebra folds both post-attention linear steps away:
  - softmax rows sum to 1  =>  attn @ (v+bv) == attn@v + bv, so the v
    bias becomes an output bias  bo' = Wo @ bv + bo.
  - (attn @ v) @ Wo.T == attn @ (t @ (Wo@Wv).T): with Wvo = Wo@Wv
    precomputed on host, the VW projection directly produces
    final-channel values; no device-side output projection.
The kernel returns outT (c, n) per core; the host transposes during
unsharding (a pure layout move).

Per-core dataflow (main matmuls bf16, f32 PSUM accumulate; softmax
normalization in exact f32):
  kT_own[c,m]  = Wk @ tq + bk      -> DRAM, pair AllGather -> kT (c,4096)
  VW_own[m,c]  = tq.T @ WvoT       -> DRAM, pair AllGather -> VW (4096,c)
  qT[c,n]      = Wq @ tq + bq      per 512-token n-chunk (tq kept in SBUF)
  ST[m,n]      = kT.T-chunks @ qT      (scores, transposed)
  P[m,n]       = exp(ST/sqrt(c))       ScalarE, no max-subtract (|scores|<~2)
  acc[m%128,n] += P                    DVE accumulate (for rowsum)
  OT[c,n]     += VW-chunk.T @ P        (PSUM-accumulated over m-tiles)
  OT[c,n]     += bo'[c-chunk] x rowsum[n]   (K=1 matmul; exact bias)
  rowsum = ones.T @ acc (f32 MM); rinv_bc = broadcast(1/rowsum) (K=1 MM)
  outT[c,n] = OT * rinv_bc             (DVE, PSUM->SBUF) -> DMA
"""

import sys

for _p in ("/opt/trn_rl_repo", "/root/.axon_site/_ro/trn_rl_repo"):
    if _p not in sys.path:
        sys.path.append(_p)

import numpy as np
import ml_dtypes

import concourse.bacc as bacc
import concourse.mybir as mybir
import concourse.tile as tile
from concourse.bass_utils import run_bass_kernel_spmd

DT = mybir.dt.float32
BF = mybir.dt.bfloat16
AFT = mybir.ActivationFunctionType

B, C, HW = 4, 512, 4096          # batch, channels, tokens per batch
NQ = HW // 2                     # q tokens per core (2048)
CK = C // 128                    # contraction chunks (4)
MT = HW // 128                   # key/value tiles over the full axis (32)
NB = NQ // 512                   # q-chunks per core (4)
JH = NQ // 512                   # token chunks in own half (4)
SCALE = 1.0 / float(np.sqrt(C))
N_CORES = 8
GROUPS = [[0, 1], [2, 3], [4, 5], [6, 7]]

_compiled = None
_ONES = np.ones(128, dtype=np.float32)


def _build():
    nc = bacc.Bacc("TRN2", target_bir_lowering=False)

    xq_e = nc.declare_dram_parameter("xq", [C, NQ], BF, isOutput=False)
    wqt_e = nc.declare_dram_parameter("wqt", [C, C], BF, isOutput=False)
    wkt_e = nc.declare_dram_parameter("wkt", [C, C], BF, isOutput=False)
    wvot_e = nc.declare_dram_parameter("wvot", [C, C], BF, isOutput=False)
    bq_e = nc.declare_dram_parameter("bq", [C], DT, isOutput=False)
    bk_e = nc.declare_dram_parameter("bk", [C], DT, isOutput=False)
    bop_e = nc.declare_dram_parameter("bop", [C], BF, isOutput=False)
    ones_e = nc.declare_dram_parameter("ones_r", [128], DT, isOutput=False)
    out_e = nc.declare_dram_parameter("outT", [C, NQ], DT, isOutput=True)

    with tile.TileContext(nc) as tc:
        with (
            tc.tile_pool(name="kt", bufs=1) as kt_pool,
            tc.tile_pool(name="vv", bufs=1) as vv_pool,
            tc.tile_pool(name="tq", bufs=1) as tq_pool,
            tc.tile_pool(name="wq", bufs=1) as wq_pool,
            tc.tile_pool(name="consts", bufs=1) as c_pool,
            tc.tile_pool(name="dram", bufs=1, space="DRAM") as dram_pool,
        ):
            # ---- persistent tiles ----
            kt_sb = [kt_pool.tile([128, HW], BF, tag=f"k{i}", name=f"k{i}") for i in range(CK)]
            vw_sb = [vv_pool.tile([128, C], BF, tag=f"v{i}", name=f"v{i}") for i in range(MT)]
            # own-half tokens, kept resident: feed kT/VW projections AND qT
            tq_sb = [tq_pool.tile([128, 512], BF, tag=f"tq{i}", name=f"tq{i}")
                     for i in range(CK * JH)]
            wq_sb = [wq_pool.tile([128, C], BF, tag=f"wq{i}", name=f"wq{i}") for i in range(CK)]

            bq_t = c_pool.tile([128, CK], DT, tag="bq", name="bq_t")
            bk_t = c_pool.tile([128, CK], DT, tag="bk", name="bk_t")
            bop_row = c_pool.tile([1, C], BF, tag="bop", name="bop_row")
            ones_col_f = c_pool.tile([128, 1], DT, tag="onescf", name="ones_col_f")
            ones_row_f = c_pool.tile([1, 128], DT, tag="onesrf", name="ones_row_f")
            for t in range(CK):
                nc.sync.dma_start(bk_t[:, t:t + 1], bk_e[t * 128:(t + 1) * 128])
            nc.sync.dma_start(ones_col_f[:, 0:1], ones_e[:])
            nc.sync.dma_start(ones_row_f[0:1, :], ones_e[:])

            # exchange buffers (pair AllGather concatenates along dim 0)
            kt_loc = dram_pool.tile([C, NQ], BF, tag="ktloc", name="kt_loc")
            vw_loc = dram_pool.tile([NQ, C], BF, tag="vwloc", name="vw_loc")
            kt_g = dram_pool.tile([2 * C, NQ], BF, tag="ktg", name="kt_g")
            vw_g = dram_pool.tile([HW, C], BF, tag="vwg", name="vw_g")

            # ---- phase 1: project own half; exchange with pair neighbor ----
            with (
                tc.tile_pool(name="wkv", bufs=1) as wkv_pool,
                tc.tile_pool(name="stg", bufs=3) as stg_pool,
                tc.tile_pool(name="ps1", bufs=2, space="PSUM") as ps1,
            ):
                wk_sb = [wkv_pool.tile([128, C], BF, tag=f"wk{i}", name=f"wk{i}") for i in range(CK)]
                wv_sb = [wkv_pool.tile([128, C], BF, tag=f"wv{i}", name=f"wv{i}") for i in range(CK)]
                for i in range(CK):
                    nc.sync.dma_start(wk_sb[i][:], wkt_e[i * 128:(i + 1) * 128, :])
                for ci in range(CK):
                    for j in range(JH):
                        nc.gpsimd.dma_start(
                            tq_sb[ci * JH + j][:],
                            xq_e[ci * 128:(ci + 1) * 128, j * 512:(j + 1) * 512],
                        )
                for i in range(CK):
                    nc.sync.dma_start(wv_sb[i][:], wvot_e[i * 128:(i + 1) * 128, :])

                # kT for own half -> kt_loc
                for j in range(JH):
                    for co in range(CK):
                        pk = ps1.tile([128, 512], DT, tag="pk", name="pk")
                        for ci in range(CK):
                            nc.tensor.matmul(
                                pk[:], wk_sb[ci][:, co * 128:(co + 1) * 128],
                                tq_sb[ci * JH + j][:], start=(ci == 0), stop=(ci == CK - 1),
                            )
                        ktst = stg_pool.tile([128, 512], BF, tag="ktst", name="ktst")
                        nc.scalar.activation(ktst[:], pk[:], AFT.Identity,
                                             bias=bk_t[:, co:co + 1])
                        nc.sync.dma_start(
                            kt_loc[co * 128:(co + 1) * 128, j * 512:(j + 1) * 512], ktst[:]
                        )
                nc.gpsimd.collective_compute(
                    "AllGather", mybir.AluOpType.bypass, replica_groups=GROUPS,
                    ins=[kt_loc.opt()], outs=[kt_g.opt()],
                )

                # VW for own half -> vw_loc
                for j in range(JH):
                    for ml in range(4):
                        pv = ps1.tile([128, 512], DT, tag="pv", name="pv")
                        for ci in range(CK):
                            nc.tensor.matmul(
                                pv[:], tq_sb[ci * JH + j][:, ml * 128:(ml + 1) * 128],
                                wv_sb[ci][:], start=(ci == 0), stop=(ci == CK - 1),
                            )
                        vwst = stg_pool.tile([128, 512], BF, tag="vwst", name="vwst")
                        nc.vector.tensor_copy(vwst[:], pv[:])
                        r0 = (4 * j + ml) * 128
                        nc.sync.dma_start(vw_loc[r0:r0 + 128, :], vwst[:])
                nc.gpsimd.collective_compute(
                    "AllGather", mybir.AluOpType.bypass, replica_groups=GROUPS,
                    ins=[vw_loc.opt()], outs=[vw_g.opt()],
                )

            # load gathered halves (both cores see identical m-order)
            for ci in range(CK):
                nc.sync.dma_start(kt_sb[ci][:, 0:NQ], kt_g[ci * 128:(ci + 1) * 128, :])
                nc.sync.dma_start(
                    kt_sb[ci][:, NQ:HW], kt_g[C + ci * 128:C + (ci + 1) * 128, :]
                )
            for mt in range(MT):
                nc.sync.dma_start(vw_sb[mt][:], vw_g[mt * 128:(mt + 1) * 128, :])

            # phase-2 weights/consts arrive while phase-1 compute runs
            for i in range(CK):
                nc.sync.dma_start(wq_sb[i][:], wqt_e[i * 128:(i + 1) * 128, :])
            for t in range(CK):
                nc.sync.dma_start(bq_t[:, t:t + 1], bq_e[t * 128:(t + 1) * 128])
            nc.sync.dma_start(bop_row[0:1, :], bop_e[:])

            # ---- phase 2: attention per 512-token q-chunk ----
            with (
                tc.tile_pool(name="qcp", bufs=2) as qc_pool,
                tc.tile_pool(name="pexp", bufs=4) as pe_pool,
                tc.tile_pool(name="accp", bufs=2) as acc_pool,
                tc.tile_pool(name="rsp", bufs=2) as rs_pool,
                tc.tile_pool(name="outp", bufs=3) as out_pool,
                tc.tile_pool(name="smallp", bufs=2) as small_pool,
                tc.tile_pool(name="ps2", bufs=3, space="PSUM") as ps2,
                tc.tile_pool(name="psot", bufs=1, space="PSUM") as psot,
                tc.tile_pool(name="psrs", bufs=1, space="PSUM") as psrs,
            ):
                for nb in range(NB):
                    # qT chunk (c, 512) from resident tq tiles
                    qcs = []
                    for co in range(CK):
                        pq = ps2.tile([128, 512], DT, tag="st", name="st")
                        for ci in range(CK):
                            nc.tensor.matmul(
                                pq[:], wq_sb[ci][:, co * 128:(co + 1) * 128],
                                tq_sb[ci * JH + nb][:], start=(ci == 0), stop=(ci == CK - 1),
                            )
                        qc = qc_pool.tile([128, 512], BF, tag=f"qc{co}", name=f"qc{co}")
                        nc.scalar.activation(qc[:], pq[:], AFT.Identity, bias=bq_t[:, co:co + 1])
                        qcs.append(qc)

                    acc = acc_pool.tile([128, 512], DT, tag="acc", name="acc")
                    ots = [psot.tile([128, 512], DT, tag=f"ot{co}", name=f"ot{co}") for co in range(CK)]
                    for mt in range(MT):
                        st = ps2.tile([128, 512], DT, tag="st", name="st")
                        for ci in range(CK):
                            nc.tensor.matmul(
                                st[:], kt_sb[ci][:, mt * 128:(mt + 1) * 128],
                                qcs[ci][:], start=(ci == 0), stop=(ci == CK - 1),
                            )
                        pexp = pe_pool.tile([128, 512], BF, tag="pe", name="pexp")
                        nc.scalar.activation(pexp[:], st[:], AFT.Exp, scale=SCALE)
                        if mt == 0:
                            nc.vector.tensor_copy(acc[:], pexp[:])
                        else:
                            nc.vector.tensor_add(acc[:], acc[:], pexp[:])
                        for co in range(CK):
                            nc.tensor.matmul(
                                ots[co][:], vw_sb[mt][:, co * 128:(co + 1) * 128],
                                pexp[:],
                                start=(mt == 0), stop=False, skip_group_check=True,
                            )

                    # rowsum row; reciprocal; broadcast via K=1 matmul (f32)
                    rs = psrs.tile([1, 512], DT, tag="rs", name="rs")
                    nc.tensor.matmul(rs[:], ones_col_f[:, 0:1], acc[:], start=True, stop=True)
                    rs_row = small_pool.tile([1, 512], BF, tag="rsrow", name="rs_row")
                    nc.scalar.activation(rs_row[:], rs[:], AFT.Copy)
                    rinv_row = small_pool.tile([1, 512], DT, tag="rinvrow", name="rinv_row")
                    nc.vector.reciprocal(rinv_row[:], rs[:])
                    rbc_ps = psrs.tile([128, 512], DT, tag="rs", name="rbc_ps")
                    nc.tensor.matmul(rbc_ps[:], ones_row_f[0:1, :], rinv_row[0:1, :],
                                     start=True, stop=True)
                    rinv_bc = rs_pool.tile([128, 512], DT, tag="rinvbc", name="rinv_bc")
                    nc.vector.tensor_copy(rinv_bc[:], rbc_ps[:])

                    # exact bias closes each accumulation group; normalize on
                    # PSUM->SBUF evac and store
                    for co in range(CK):
                        nc.tensor.matmul(
                            ots[co][:], bop_row[0:1, co * 128:(co + 1) * 128],
                            rs_row[0:1, :], start=False, stop=True, skip_group_check=True,
                        )
                        oc = out_pool.tile([128, 512], DT, tag="oc", name="oc")
                        nc.vector.tensor_mul(oc[:], ots[co][:], rinv_bc[:])
                        nc.sync.dma_start(
                            out_e[co * 128:(co + 1) * 128, nb * 512:(nb + 1) * 512], oc[:]
                        )

    nc.compile()
    return nc


def _get_compiled():
    global _compiled
    if _compiled is None:
        _compiled = _build()
    return _compiled


def kernel(**inputs):
    x = np.ascontiguousarray(np.asarray(inputs["x"], dtype=np.float32))
    wq = np.asarray(inputs["Wq"], dtype=np.float32)
    wk = np.asarray(inputs["Wk"], dtype=np.float32)
    wv = np.asarray(inputs["Wv"], dtype=np.float32)
    wo = np.asarray(inputs["Wo"], dtype=np.float32)
    bq = np.ascontiguousarray(np.asarray(inputs["bq"], dtype=np.float32))
    bk = np.ascontiguousarray(np.asarray(inputs["bk"], dtype=np.float32))
    bv = np.asarray(inputs["bv"], dtype=np.float32)
    bo = np.asarray(inputs["bo"], dtype=np.float32)

    wqt = np.ascontiguousarray(wq.T.astype(ml_dtypes.bfloat16))
    wkt = np.ascontiguousarray(wk.T.astype(ml_dtypes.bfloat16))
    wvot = np.ascontiguousarray((wo @ wv).T.astype(ml_dtypes.bfloat16))
    bop = np.ascontiguousarray((wo @ bv + bo).astype(ml_dtypes.bfloat16))

    xb = x.reshape(B, C, HW).astype(ml_dtypes.bfloat16)
    in_maps = []
    for core in range(N_CORES):
        bi, h = core // 2, core % 2
        in_maps.append({
            "xq": np.ascontiguousarray(xb[bi][:, h * NQ:(h + 1) * NQ]),
            "wqt": wqt, "wkt": wkt, "wvot": wvot,
            "bq": bq, "bk": bk, "bop": bop, "ones_r": _ONES,
        })

    nc = _get_compiled()
    res = run_bass_kernel_spmd(nc, in_maps, core_ids=list(range(N_CORES)))

    out = np.empty((B, HW, C), dtype=np.float32)
    for core in range(N_CORES):
        bi, h = core // 2, core % 2
        out[bi, h * NQ:(h + 1) * NQ, :] = res.results[core]["outT"].T
    return out.reshape(B, C, 64, 64)
